# revision 32
# baseline (speedup 1.0000x reference)
"""Trainium2 Bass kernel for BiDAF-style bidirectional attention (v8).

Reference computation (per batch element n; M=1 folded away):
    s[i,j]  = h[i].w_h + u[j].w_u + (h[i]*u[j]).w_hu + b      [JX, JQ]
    a_u     = softmax_j(s);     u_a[i] = sum_j a_u[i,j] u[j]   (c2q)
    a_h     = softmax_i(max_j s);  h_a = sum_i a_h[i] h[i]     (q2c)
    out     = concat(h, u_a, h*u_a, h*h_a)                     [JX, 4D]

Sharding: data-parallel over batch N=8, one NeuronCore per batch element.
alpha_b drops out (both softmaxes are shift-invariant); accepted but unused.

v8 design (bf16 I/O, host-folded weights -- see _prep_inputs):
  - EVERY DMA job is a whole, DRAM-contiguous tensor (strided jobs measured
    ~2-3x slower): hT ships as 8 per-(block,chunk) tensors so each score
    matmul gates on its own 128KB chunk; h and the output slabs ship per
    512-row half.
  - Input queues: hT chunks on Sync, h halves on GpSimd (dep-gated behind
    warmup mm#3 so they never starve the hT stream), uwbT/u on Scalar.
  - 4-matmul bf16 warmup bridges PE start to the first chunk's arrival;
    scores then hold the PE busy so the HAM gate opens ~11.5us.
  - ET = exp(sT + uwu[j]) via ACT bias; c2q u_a through a 4-deep PSUM
    rotation; o2 = u_a/z split Scalar x6 / DVE x2; o3 = o2*h as 2-tile
    bf16 2x TTs on DVE only (DVE+GpSimd share SBUF ports).
  - q2c: 8 ET re-transposes in one PSUM bank -> per-block j-max on DVE;
    zsum via N=1 PE matmuls; mrow/mrow_bf on GpSimd; zq/haT on PE; ha_row
    on Scalar; o4T[d,i] = hT * hacol via 4x DVE tensor_scalar; the whole
    q2c tail chain runs under tc.high_priority() so the list scheduler
    doesn't push it behind the evictions.
"""

import numpy as np

N_B, M_B, JX, JQ, D = 8, 1, 1024, 128, 512
P = 128
NT = JX // P    # 8 i-tiles
KC = D // P     # 4 d-chunks
IB = 512        # i-block width for score matmuls
NB = JX // IB   # 2 blocks
TPB = NT // NB  # 4 tiles per block

_CACHE = {}


def _build_program():
    from contextlib import ExitStack

    import concourse.bass as bass
    import concourse.tile as tile
    from concourse import bacc, mybir
    from concourse.masks import make_identity
    from concourse.tile_rust import add_dep_helper

    f32 = mybir.dt.float32
    bf16 = mybir.dt.bfloat16
    EXP = mybir.ActivationFunctionType.Exp
    AX = mybir.AxisListType.X
    ds = bass.ds

    nc = bacc.Bacc("TRN2", target_bir_lowering=False, debug=False, num_devices=8)
    # input jobs sized for the sync-queue priority stream: fine chunks up
    # front (they gate the first scores), bigger jobs after
    hc_d = [nc.dram_tensor("hT00", [P, IB], bf16, kind="ExternalInput").ap(),
            nc.dram_tensor("hT01", [P, IB], bf16, kind="ExternalInput").ap(),
            nc.dram_tensor("hT023", [P, 2 * IB], bf16, kind="ExternalInput").ap(),
            nc.dram_tensor("hT101", [P, 2 * IB], bf16, kind="ExternalInput").ap(),
            nc.dram_tensor("hT123", [P, 2 * IB], bf16, kind="ExternalInput").ap()]
    h_d = [nc.dram_tensor(f"h{b}", [P, TPB * D], bf16, kind="ExternalInput").ap()
           for b in range(NB)]
    uwbT_d = nc.dram_tensor("uwbT", [P, D], bf16, kind="ExternalInput").ap()
    u_d = nc.dram_tensor("u", [P, D], bf16, kind="ExternalInput").ap()
    uwu_d = nc.dram_tensor("uwu", [P, 1], f32, kind="ExternalInput").ap()
    # outputs per (slab, half): 0=h rows, 1=u_a rows, 2=h*u_a rows, 3=o4T
    o_d = [[nc.dram_tensor(f"o{s}{b}", [P, TPB * D], bf16, kind="ExternalOutput").ap()
            for b in range(NB)] for s in range(4)]

    with tile.TileContext(nc) as tc, ExitStack() as ctx:
        consts = ctx.enter_context(tc.tile_pool(name="consts", bufs=1))
        stage = ctx.enter_context(tc.tile_pool(name="stage", bufs=1))
        # PSUM budget (8 banks): acc=1, s0=2(reused), tp=1, ua=2, ux=1, hap=1
        ps = ctx.enter_context(tc.tile_pool(name="ps", bufs=2, space="PSUM"))

        # ---- input DMAs: ONE sync-queue stream in strict priority order
        # (single queue => jobs transfer in order; no cross-competition) ----
        aux = consts.tile([P, 2 * D], bf16)
        u_sb = aux[:, ds(0, D)]
        uwbT = aux[:, ds(D, D)]
        hT = consts.tile([P, NB * KC * IB], bf16)   # block-major
        h_all = consts.tile([P, NT * D], bf16)      # tile t: h[t*128+p, d]
        nc.sync.dma_start(aux[:, ds(D, D)], uwbT_d[:])
        nc.sync.dma_start(hT[:, ds(0, IB)], hc_d[0][:])
        nc.sync.dma_start(hT[:, ds(IB, IB)], hc_d[1][:])
        nc.sync.dma_start(hT[:, ds(2 * IB, 2 * IB)], hc_d[2][:])
        nc.sync.dma_start(hT[:, ds(KC * IB, 2 * IB)], hc_d[3][:])
        nc.sync.dma_start(hT[:, ds((KC + 2) * IB, 2 * IB)], hc_d[4][:])
        nc.sync.dma_start(aux[:, ds(0, D)], u_d[:])
        for b in range(NB):
            nc.sync.dma_start(h_all[:, ds(b * TPB * D, TPB * D)], h_d[b][:])
        uwu = consts.tile([P, 1], f32)
        nc.gpsimd.dma_start(uwu[:], uwu_d[:])
        ident = consts.tile([P, P], bf16)
        make_identity(nc, ident[:])                 # gpsimd affine_select

        # ---- constants ----
        warm = consts.tile([P, D], bf16)
        nc.vector.memset(warm[:], 0.25)
        ones_col = consts.tile([P, 1], bf16)
        nc.vector.memset(ones_col[:], 1.0)
        one1 = consts.tile([1, 1], bf16)
        nc.vector.memset(one1[:], 1.0)

        # ---- PE warmup bridges to the first hT chunk's arrival ----
        wp = ps.tile([P, D], f32, tag="acc", bufs=1)
        warm_mms = [
            nc.tensor.matmul(wp[:], warm[:, ds(0, P)], warm[:], start=True, stop=True)
            for _ in range(4)
        ]

        # ---- working tiles ----
        ET = consts.tile([JQ, JX], bf16)
        m_exp = consts.tile([P, NT], f32)
        m_bf = consts.tile([P, NT], bf16)
        z_rec = consts.tile([P, NT], f32)
        hap = ps.tile([1, D], f32, tag="hap", bufs=1)
        ua_blk = [
            stage.tile([P, TPB * D], bf16, tag=f"ua{b}", name=f"ua_blk{b}")
            for b in range(NB)
        ]
        o3_blk = [
            stage.tile([P, TPB * D], bf16, tag=f"o3{b}", name=f"o3_blk{b}")
            for b in range(NB)
        ]
        o4T = consts.tile([P, KC * JX], bf16)       # chunk-major

        # ---- scores + exp per block (each matmul gates on its chunk).
        # Short N=128 fillers between chunk matmuls keep the PE busy across
        # chunk-arrival gaps so the HAM clock gate opens early. ----
        sps = []
        for b in range(NB):
            sp = ps.tile([JQ, IB], f32, tag="s0")
            for k in range(KC):
                nc.tensor.matmul(
                    sp[:], uwbT[:, ds(k * JQ, JQ)],
                    hT[:, ds((b * KC + k) * IB, IB)],
                    start=(k == 0), stop=(k == KC - 1),
                )
                if b == 0 and k < KC - 1:
                    for _ in range(3):
                        nc.tensor.matmul(
                            wp[:, ds(0, P)], warm[:, ds(0, P)], warm[:, ds(0, P)],
                            start=True, stop=True,
                        )
            sps.append(sp)
        for b in range(NB):
            nc.scalar.activation(ET[:, ds(b * IB, IB)], sps[b][:], EXP, bias=uwu[:])

        # ---- passthrough (slab 0) per half as h lands ----
        for b in range(NB):
            nc.gpsimd.dma_start(o_d[0][b][:], h_all[:, ds(b * TPB * D, TPB * D)])

        # ---- ET re-transposes (one PSUM bank) + per-block max/zsum ----
        zcol = ps.tile([P, NT], f32, tag="acc", bufs=1)
        et = ps.tile([P, NT * P], bf16, tag="tp", bufs=1)
        for b in range(NB):
            for q in range(TPB):
                t = b * TPB + q
                nc.tensor.transpose(et[:, ds(t * P, P)], ET[:, ds(t * P, P)], ident[:])
            for q in range(TPB):
                t = b * TPB + q
                nc.tensor.matmul(
                    zcol[:, ds(t, 1)], ET[:, ds(t * P, P)], ones_col[:],
                    start=True, stop=True, skip_group_check=True,
                )
            sl = ds(b * TPB, TPB)
            nc.vector.reciprocal(z_rec[:, sl], zcol[:, sl])
            et3 = et[:].rearrange("p (t x) -> p t x", t=NT)
            nc.vector.reduce_max(m_exp[:, sl], et3[:, ds(b * TPB, TPB), :], axis=AX)
            nc.gpsimd.tensor_copy(m_bf[:, sl], m_exp[:, sl])

        # ---- c2q u_a (4-deep PSUM rotation); q2c hap/zq ----
        up_tags = ["ua", "ua", "s0", "s0", "ua", "ua", "ux", "s0"]
        ups = []
        for t in range(TPB):
            up = ps.tile([P, D], f32, tag=up_tags[t], name=f"up{t}")
            nc.tensor.matmul(up[:], ET[:, ds(t * P, P)], u_sb, start=True, stop=True)
            ups.append(up)
        for t in range(NT):
            nc.tensor.matmul(
                hap[:], m_bf[:, ds(t, 1)], h_all[:, ds(t * D, D)],
                start=(t == 0), stop=(t == NT - 1), skip_group_check=True,
            )
        with tc.high_priority():
            mrow = consts.tile([P, 1], f32)
            nc.vector.reduce_sum(mrow[:], m_exp[:], axis=AX)
            mrow_bf = consts.tile([P, 1], bf16)
            nc.vector.tensor_copy(mrow_bf[:], mrow[:])
            zqp = ps.tile([1, 1], f32, tag="acc", bufs=1)
            nc.tensor.matmul(zqp[:], mrow_bf[:], ones_col[:], start=True, stop=True)
        for t in range(TPB, NT):
            up = ps.tile([P, D], f32, tag=up_tags[t], name=f"up{t}",
                         bufs=1 if t == 6 else 2)
            nc.tensor.matmul(up[:], ET[:, ds(t * P, P)], u_sb, start=True, stop=True)
            ups.append(up)

        # ---- evictions + q2c tail ----
        def o3_pair(pair):
            b, qp = divmod(pair, TPB // 2)
            nc.vector.tensor_mul(
                o3_blk[b][:, ds(qp * 2 * D, 2 * D)],
                ua_blk[b][:, ds(qp * 2 * D, 2 * D)],
                h_all[:, ds(pair * 2 * D, 2 * D)],
            )

        def o2(t, eng):
            b, q = divmod(t, TPB)
            dst = ua_blk[b][:, ds(q * D, D)]
            if eng is nc.scalar:
                nc.scalar.mul(dst, ups[t][:], z_rec[:, ds(t, 1)])
            else:
                eng.tensor_scalar_mul(dst, ups[t][:], z_rec[:, ds(t, 1)])

        o2(0, nc.scalar)
        o2(1, nc.scalar)
        o3_pair(0)                       # vector, fires when o2 t1 done
        with tc.high_priority():
            rzq = consts.tile([1, 1], f32)
            nc.vector.reciprocal(rzq[:], zqp[:])
            ha_row = consts.tile([1, D], bf16)
            nc.scalar.mul(ha_row[:], hap[:], rzq[:])
            haT = ps.tile([P, KC], f32, tag="acc", bufs=1)
            for k in range(KC):
                nc.tensor.matmul(
                    haT[:, ds(k, 1)], ha_row[:, ds(k * P, P)], one1[:],
                    start=True, stop=True, skip_group_check=True,
                )
        o2(2, nc.scalar)
        o2(3, nc.scalar)
        o2(5, nc.vector)
        o2(7, nc.vector)
        o3_pair(1)                       # vector
        nc.sync.dma_start(o_d[1][0][:], ua_blk[0][:])
        nc.sync.dma_start(o_d[2][0][:], o3_blk[0][:])
        o2(4, nc.scalar)
        o2(6, nc.scalar)
        # vector: hacol then o4T (4x tensor_scalar, 3D APs) then o3 b1
        with tc.high_priority():
            hacol = consts.tile([P, KC], f32)
            nc.vector.tensor_copy(hacol[:], haT[:])
            hT4 = hT[:].rearrange("p (b k x) -> p b k x", b=NB, k=KC)
            o4T4 = o4T[:].rearrange("p (k b x) -> p k b x", k=KC, b=NB)
            for k in range(KC):
                nc.vector.tensor_scalar_mul(
                    o4T4[:, k], hT4[:, :, k, :], hacol[:, ds(k, 1)]
                )
            nc.gpsimd.dma_start(o_d[3][0][:], o4T[:, ds(0, 2 * JX)])
            nc.scalar.dma_start(o_d[3][1][:], o4T[:, ds(2 * JX, 2 * JX)])
        o3_pair(2)                       # vector
        nc.sync.dma_start(o_d[1][1][:], ua_blk[1][:])
        o3_pair(3)                       # vector
        nc.sync.dma_start(o_d[2][1][:], o3_blk[1][:])

    nc.compile()
    return nc


def _get_nc():
    if "nc" not in _CACHE:
        _CACHE["nc"] = _build_program()
    return _CACHE["nc"]


def _ensure_axon_hooks_stub():
    import sys
    import types

    try:
        import antenv.axon_hooks  # noqa: F401
    except ImportError:
        mod = types.ModuleType("antenv.axon_hooks")
        _hook = [None]
        mod.set_axon_ntff_profile_hook = lambda hook: _hook.__setitem__(0, hook)
        mod.get_axon_ntff_profile_hook = lambda: _hook[0]
        sys.modules["antenv.axon_hooks"] = mod


def _prep_inputs(h, u, alpha_w):
    """Host-side layout/weight prep (data movement + O(JQ*D) weight folding)."""
    import ml_dtypes

    bf = ml_dtypes.bfloat16
    w_h, w_u, w_hu = alpha_w[:D], alpha_w[D:2 * D], alpha_w[2 * D:]
    in_maps = []
    for n in range(N_B):
        hn = h[n]                                   # [JX, D] f32
        un = u[n]                                   # [JQ, D] f32
        hrows = hn.reshape(NB, TPB, P, D).transpose(0, 2, 1, 3)  # [NB,P,TPB,D]
        # hT chunks: [p, i'] = h[b*IB+i', k*128+p]
        hTc = hn.T.reshape(KC, P, NB, IB)
        m = {}
        for b in range(NB):
            m[f"h{b}"] = np.ascontiguousarray(
                hrows[b].reshape(P, TPB * D)).astype(bf)
        m["hT00"] = np.ascontiguousarray(hTc[0, :, 0, :]).astype(bf)
        m["hT01"] = np.ascontiguousarray(hTc[1, :, 0, :]).astype(bf)
        m["hT023"] = np.ascontiguousarray(
            hTc[2:4, :, 0, :].transpose(1, 0, 2).reshape(P, 2 * IB)).astype(bf)
        m["hT101"] = np.ascontiguousarray(
            hTc[0:2, :, 1, :].transpose(1, 0, 2).reshape(P, 2 * IB)).astype(bf)
        m["hT123"] = np.ascontiguousarray(
            hTc[2:4, :, 1, :].transpose(1, 0, 2).reshape(P, 2 * IB)).astype(bf)
        uwb = un * w_hu[None, :] + w_h[None, :]     # [JQ, D]
        uwbT = uwb.T.reshape(KC, P, JQ).transpose(1, 0, 2).reshape(P, KC * JQ)
        m["uwbT"] = np.ascontiguousarray(uwbT).astype(bf)
        m["u"] = np.ascontiguousarray(un).astype(bf)
        m["uwu"] = (un @ w_u).reshape(P, 1).astype(np.float32)
        in_maps.append(m)
    return in_maps


def _decode_out(res):
    outs = []
    for n in range(N_B):
        r = {k: np.asarray(v).astype(np.float32) for k, v in res.results[n].items()}
        # slabs 0-2: [P, TPB, D] per half -> rows
        rows = np.stack(
            [np.concatenate([r[f"o{s}0"], r[f"o{s}1"]], axis=1) for s in range(3)],
            axis=2,
        )  # [P, 2*TPB*D, 3] -> careful: axis1 = NT*D
        rows = rows.reshape(P, NT, D, 3).transpose(1, 0, 3, 2)   # [NT, P, 3, D]
        # slab 3: o4T chunk-major [P, KC*JX] -> o4[i, k*128+p]
        o4T = np.concatenate([r["o30"], r["o31"]], axis=1)
        o4 = o4T.reshape(P, KC, JX).transpose(2, 1, 0)           # [JX, KC, P]
        full = np.concatenate(
            [rows.reshape(JX, 3 * D), o4.reshape(JX, D)], axis=1
        )
        outs.append(full)
    return np.stack(outs, axis=0).reshape(N_B, M_B, JX, 4 * D)


def kernel(h, u, alpha_w, alpha_b=None, **_unused):
    _ensure_axon_hooks_stub()
    from concourse.bass_utils import run_bass_kernel_spmd

    h = np.ascontiguousarray(np.asarray(h, dtype=np.float32)).reshape(N_B, JX, D)
    u = np.ascontiguousarray(np.asarray(u, dtype=np.float32)).reshape(N_B, JQ, D)
    alpha_w = np.ascontiguousarray(np.asarray(alpha_w, dtype=np.float32)).reshape(3 * D)

    nc = _get_nc()
    in_maps = _prep_inputs(h, u, alpha_w)
    res = run_bass_kernel_spmd(nc, in_maps, core_ids=list(range(N_B)))
    return _decode_out(res)
